# revision 10
# baseline (speedup 1.0000x reference)
"""Distributed Trainium2 kernel for causal multi-head attention with LoRA
(c_attn + c_proj both LoRA'd), B=2 T=2048 C=1024 H=16 hd=64 r=8.

Sharding: data-parallel over batch (2 groups of 4 cores) x tensor-parallel
over heads (4 heads / core).  Each core computes qkv for its heads, causal
attention, and a partial c_proj over its 256 input dims; the host sums the
4 partials per batch (collectives pay ~40us ncfw init here).

Host-side simplifications (all exact linear algebra, no approximation):
 - LoRA folds into the base weights: W_eff = W + LORA_SCALE * B @ A.
 - Everything is passed feature-major ("pre-transposed") so no on-device
   transposes are needed; the device output is y^T, transposed back on host.
 - b_attn / b_proj are zeros by the problem spec and are not applied.

Device compute is bf16 (fp32 PSUM accumulation; rel-err budget 2e-2).

Schedule (single merged emission stream, engines pipeline via Tile deps):
 - qkv production is t-chunk-major (A(ci)), and attention q-chunks are
   processed in causal order (256,512,1024,1536,0) so attention's exp
   (ScalarE-bound) overlaps qkv GEMMs (PE-bound) instead of serializing.
 - A-quanta (one qk j-tile / v t-tile GEMM chain) and c_proj m-tiles of the
   previous chunk are woven between attention windows from a filler queue,
   keeping the PE dense so the HAM clock gate stays at 8/8.
 - S^T tiles ([k,q] layout): the two heads of a pair are emitted
   interleaved so their K=64 matmuls row-pack (tile_position rows 0/64)
   and run concurrently on the PE.  Fully-masked columns (q < 128j on
   diagonal tiles) are skipped in both the S matmuls and the exp.
 - exp on ScalarE (scale=1/8 folded in; no max-subtraction: |logits|<~4,
   fp32 exp overflows at 88) -> causal 0/1 mask multiply on the 128-col
   diagonal block only -> PV matmul with V augmented by a ones column so
   softmax denominators fall out of the same matmul (psum row 64).
 - O is copied out unnormalized (frees PSUM); denominators are
   reciprocal'd on a packed [4, qw] tile and broadcast to the 64 dims of
   each head with a tiny K=4 one-hot matmul (E4).
"""

import numpy as np
import ml_dtypes

import concourse.bass as bass
import concourse.mybir as mybir
import concourse.tile as tile
from concourse import bacc

BF16 = mybir.dt.bfloat16
F32 = mybir.dt.float32
NPBF = ml_dtypes.bfloat16

B, T, C = 2, 2048, 1024
H, HD, R = 16, 64, 8
LORA_SCALE = 2.0

TP = 4                 # tensor-parallel ranks per batch group
HL = H // TP           # heads per core = 4
OQ = HL * HD           # local q rows = 256
OL = 3 * OQ            # local qkv rows = 768
CP = C // TP           # local c_proj contraction dims = 256
TC = 512               # t-chunk (matmul free dim)
NTC = T // TC          # 4 chunks
KT = 128               # k tile (partition dim of S^T)
NCT = C // 128         # 8 contraction tiles for c_attn

# attention q-chunks in causal-availability order; (q0, qw, ci_needed)
CHUNKS = [(256, 256, 0), (512, 512, 1), (1024, 512, 2), (1536, 512, 3),
          (0, 256, 0)]
N_WARM = 28            # PE warmup matmuls (cover the input-DMA window)


def build_nc():
    nc = bacc.Bacc(None, target_bir_lowering=False)

    xt_d = nc.declare_dram_parameter("xt", [C, T], BF16, isOutput=False)
    wqkvt_d = nc.declare_dram_parameter("wqkvt", [C, OL], BF16, isOutput=False)
    wpt_d = nc.declare_dram_parameter("wpt", [CP, C], BF16, isOutput=False)
    masks_d = nc.declare_dram_parameter("masks", [4, KT, TC], BF16, isOutput=False)
    e4_d = nc.declare_dram_parameter("e4", [128, 2 * KT], BF16, isOutput=False)
    out_d = nc.declare_dram_parameter("out", [C, T], BF16, isOutput=True)

    with tile.TileContext(nc) as tc:
        with (
            tc.tile_pool(name="const", bufs=1) as const,
            tc.tile_pool(name="work", bufs=3) as work,
            tc.tile_pool(name="ps_lin", bufs=2, space="PSUM") as ps_lin,
            tc.tile_pool(name="ps_s", bufs=2, space="PSUM") as ps_s,
            tc.tile_pool(name="ps_o", bufs=1, space="PSUM") as ps_o,
        ):
            # ---------------- persistent SBUF tensors ----------------
            warm_s = const.tile([128, TC], BF16, tag="warm")
            nc.gpsimd.memset(warm_s, 0.0)
            warm_ps = ps_o.tile([128, TC], F32, tag="o0", name="warm_ps")
            for _ in range(N_WARM):
                nc.tensor.matmul(
                    warm_ps, lhsT=warm_s[:, :128], rhs=warm_s,
                    start=True, stop=True,
                )

            wq_s = const.tile([128, NCT, OL], BF16, tag="wq")
            wq_r = wqkvt_d.rearrange("(n p) o -> p n o", p=128)
            for n in range(NCT):
                nc.sync.dma_start(out=wq_s[:, n, :], in_=wq_r[:, n, :])

            xt_s = const.tile([128, NCT, T], BF16, tag="xt")
            xt_r = xt_d.rearrange("(n p) t -> p n t", p=128)
            for ci in range(NTC):
                for n in range(NCT):
                    nc.sync.dma_start(
                        out=xt_s[:, n, bass.ts(ci, TC)],
                        in_=xt_r[:, n, bass.ts(ci, TC)],
                    )
                if ci == 0:
                    mask_s = const.tile([128, 4, TC], BF16, tag="mask")
                    nc.sync.dma_start(
                        out=mask_s, in_=masks_d.rearrange("j p q -> p j q")
                    )
                    e4_s = const.tile([128, 2 * KT], BF16, tag="e4")
                    nc.sync.dma_start(out=e4_s, in_=e4_d[:, :])

            wpt_s = const.tile([128, CP // 128, C], BF16, tag="wpt")
            nc.sync.dma_start(out=wpt_s, in_=wpt_d.rearrange("(n p) o -> p n o", p=128))

            # q,k feature-major: tiles 0,1 = q (256 rows), 2,3 = k
            qkvt_s = const.tile([128, 4, T], BF16, tag="qkvt")
            # v token-major, augmented: per t-tile, 4 heads x (64 dims + ones)
            v_s = const.tile([128, T // 128, HL * (HD + 1)], BF16, tag="v")
            nc.vector.memset(v_s, 1.0)  # ones columns survive the V copies
            ot_s = const.tile([128, CP // 128, T], BF16, tag="ot")
            # softmax reciprocal staging: live rows at partitions 32h; the
            # zero rows of e4 null out the stale lanes in the broadcast matmul
            recip_s = const.tile([128, TC], BF16, tag="recip")
            nc.vector.memset(recip_s, 0.0)

            # ---------------- filler quanta (PE work to weave in) -----------
            def qk_quantum(ci, j):
                def emit():
                    osl = bass.ts(j, 128)
                    tsl = bass.ts(ci, TC)
                    qk_ps = ps_lin.tile([128, TC], F32, tag="lin", name="qk_ps")
                    for n in range(NCT):
                        nc.tensor.matmul(
                            qk_ps, lhsT=wq_s[:, n, osl], rhs=xt_s[:, n, tsl],
                            start=(n == 0), stop=(n == NCT - 1),
                        )
                    nc.vector.tensor_copy(qkvt_s[:, j, tsl], qk_ps)
                return emit

            def v_quantum(tt):
                def emit():
                    v_ps = ps_lin.tile([128, TC], F32, tag="lin", name="v_ps")
                    ttsl = bass.ts(tt, 128)
                    for n in range(NCT):
                        nc.tensor.matmul(
                            v_ps[:, :OQ], lhsT=xt_s[:, n, ttsl],
                            rhs=wq_s[:, n, 2 * OQ:OL],
                            start=(n == 0), stop=(n == NCT - 1),
                        )
                    dst = v_s[:, tt, :].rearrange(
                        "p (h e) -> p h e", e=HD + 1)[:, :, 0:HD]
                    nc.vector.tensor_copy(
                        dst, v_ps[:, :OQ].rearrange("p (h e) -> p h e", e=HD)
                    )
                return emit

            # filler queue: (ci_tag, emit_fn); ci_tag = qkv chunk this quantum
            # belongs to (for barrier drains), or None for np work.
            fillers = []
            for ci in range(NTC):
                for j in range(4):
                    fillers.append((ci, qk_quantum(ci, j)))
                for tt in range(4 * ci, 4 * ci + 4):
                    fillers.append((ci, v_quantum(tt)))

            def pop_filler(k):
                for _ in range(k):
                    if fillers:
                        fillers.pop(0)[1]()

            def drain_through(ci):
                # emit every queued quantum up to and including the last one
                # tagged <= ci (np fillers in between run too; they're ready)
                last = -1
                for i, (tag, _) in enumerate(fillers):
                    if tag is not None and tag <= ci:
                        last = i
                if last >= 0:
                    for _, emit in fillers[:last + 1]:
                        emit()
                    del fillers[:last + 1]

            # ---------------- attention ----------------
            def attn_chunk(q0, qw):
                tsl = slice(q0, q0 + qw)
                kt0 = q0 // 128
                nkt = kt0 + qw // 128   # causal k-tiles for this chunk
                sums4 = work.tile([128, TC], F32, tag="sums", name="sums",
                                  bufs=2)
                for p in range(2):          # head pairs (2p, 2p+1)
                    o_ps = [
                        ps_o.tile([128, TC], F32, tag=f"o{h01}", name=f"o{h01}")
                        for h01 in range(2)
                    ]
                    for w in range(nkt // 2):   # windows of 2 k-tiles
                        # S matmuls, h0/h1 interleaved so the two K=64
                        # matmuls row-pack (rows 0-63 / 64-127) on the PE
                        s_ps = [
                            ps_s.tile([128, 2 * TC], F32, tag=f"s{h01}",
                                      name=f"s{h01}", bufs=1)
                            for h01 in range(2)
                        ]
                        qls = []
                        for kt01 in range(2):
                            kt = 2 * w + kt01
                            ql = max(0, 128 * (kt - kt0))
                            qls.append(ql)
                            for h01 in range(2):
                                dsl = slice(64 * h01, 64 * h01 + 64)
                                nc.tensor.matmul(
                                    s_ps[h01][:, kt01 * qw + ql:(kt01 + 1) * qw],
                                    lhsT=qkvt_s[dsl, 2 + p, bass.ts(kt, KT)],
                                    rhs=qkvt_s[dsl, p, q0 + ql:q0 + qw],
                                    start=True, stop=True,
                                )
                        ql0, ql1 = qls
                        for h01 in range(2):
                            h = 2 * p + h01
                            pt = work.tile(
                                [128, 2 * TC], BF16, tag=f"pt{h01}",
                                name=f"pt{h01}", bufs=6,
                            )
                            if ql1 >= 352:
                                # two exp calls beat one (352-cycle ACT
                                # instruction overhead crossover)
                                for kt01, ql in enumerate(qls):
                                    nc.scalar.activation(
                                        pt[:, kt01 * qw + ql:(kt01 + 1) * qw],
                                        s_ps[h01][:, kt01 * qw + ql:(kt01 + 1) * qw],
                                        mybir.ActivationFunctionType.Exp,
                                        scale=0.125,
                                    )
                            else:
                                # single call; the dead gap [qw, qw+ql1) is
                                # exp'd on stale psum and never read
                                nc.scalar.activation(
                                    pt[:, ql0:2 * qw],
                                    s_ps[h01][:, ql0:2 * qw],
                                    mybir.ActivationFunctionType.Exp,
                                    scale=0.125,
                                )
                            # causal masking: only the 128-col diagonal block
                            # of diagonal tiles is partially masked
                            for kt01 in range(2):
                                j = 2 * w + kt01 - kt0
                                if j >= 0:
                                    c0 = 128 * j
                                    c1 = min(c0 + 128, qw)
                                    nc.vector.tensor_mul(
                                        pt[:, kt01 * qw + c0:kt01 * qw + c1],
                                        pt[:, kt01 * qw + c0:kt01 * qw + c1],
                                        mask_s[:, j, c0:c1],
                                    )
                            for kt01 in range(2):
                                kt = 2 * w + kt01
                                ql = qls[kt01]
                                nc.tensor.matmul(
                                    o_ps[h01][: HD + 1, ql:qw],
                                    lhsT=v_s[:, kt, h * (HD + 1):(h + 1) * (HD + 1)],
                                    rhs=pt[:, kt01 * qw + ql:(kt01 + 1) * qw],
                                    start=(kt == 0),
                                    stop=(kt == nkt - 1),
                                )
                        pop_filler(2)
                    # copy O out unnormalized (frees psum); gather denominators
                    for h01 in range(2):
                        h = 2 * p + h01
                        nc.vector.tensor_copy(
                            ot_s[64 * h01:64 * h01 + 64, p, tsl],
                            o_ps[h01][0:HD, :qw],
                        )
                        nc.vector.tensor_copy(
                            sums4[32 * h:32 * h + 1, :qw],
                            o_ps[h01][HD:HD + 1, :qw],
                        )
                return sums4

            # ---------------- normalize + c_proj (as filler quanta) ---------
            def np_pre(q0, qw, sums4, yt_sb, tail):
                def emit():
                    tsl = slice(q0, q0 + qw)
                    with nc.allow_low_precision(reason="softmax denom, 2e-2 budget"):
                        for h in range(HL):
                            nc.vector.reciprocal(
                                recip_s[32 * h:32 * h + 1, :qw],
                                sums4[32 * h:32 * h + 1, :qw],
                            )
                    for p in range(2):
                        rb_ps = ps_lin.tile([128, TC], F32, tag="lin",
                                            name="rb_ps")
                        nc.tensor.matmul(
                            rb_ps[:, :qw], lhsT=e4_s[:, bass.ts(p, 128)],
                            rhs=recip_s[:, :qw], start=True, stop=True,
                        )
                        dst = ot_s[:, p, tsl]
                        nc.vector.tensor_mul(dst, dst, rb_ps[:, :qw])
                return emit

            def np_mtile(q0, qw, m, yt_sb, tail):
                def emit():
                    tsl = slice(q0, q0 + qw)
                    msl = bass.ts(m, 128)
                    y_ps = ps_lin.tile([128, TC], F32, tag="lin", name="y_ps")
                    for n in range(CP // 128):
                        nc.tensor.matmul(
                            y_ps[:, :qw], lhsT=wpt_s[:, n, msl],
                            rhs=ot_s[:, n, tsl],
                            start=(n == 0), stop=(n == CP // 128 - 1),
                        )
                    if tail:
                        nc.scalar.copy(yt_sb[:, m, :qw], y_ps[:, :qw])
                    else:
                        nc.vector.tensor_copy(yt_sb[:, m, :qw], y_ps[:, :qw])
                    if m in (3, 7):
                        out_r = out_d.rearrange("(m p) t -> p m t", p=128)
                        nc.sync.dma_start(
                            out=out_r[:, m - 3:m + 1, tsl],
                            in_=yt_sb[:, m - 3:m + 1, :qw],
                        )
                return emit

            def queue_np(q0, qw, sums4, tail=False):
                yt_sb = work.tile([128, C // 128, TC], BF16, tag="yt", bufs=2,
                                  name="yt")
                fillers.append((None, np_pre(q0, qw, sums4, yt_sb, tail)))
                for m in range(C // 128):
                    fillers.append((None, np_mtile(q0, qw, m, yt_sb, tail)))

            # ---------------- main schedule ----------------
            for idx, (q0, qw, ci_need) in enumerate(CHUNKS):
                drain_through(ci_need)
                sums4 = attn_chunk(q0, qw)
                queue_np(q0, qw, sums4, tail=(idx >= 3))
            for _, emit in fillers:
                emit()
            fillers.clear()

    return nc


# ---------------- host side ----------------

def _bf(a):
    return np.ascontiguousarray(np.asarray(a, dtype=np.float32).astype(NPBF))


def make_in_maps(inputs):
    x = np.asarray(inputs["x"], np.float32)
    W_attn = np.asarray(inputs["W_attn"], np.float32)
    A_attn = np.asarray(inputs["A_attn"], np.float32)
    B_attn = np.asarray(inputs["B_attn"], np.float32)
    W_proj = np.asarray(inputs["W_proj"], np.float32)
    A_proj = np.asarray(inputs["A_proj"], np.float32)
    B_proj = np.asarray(inputs["B_proj"], np.float32)
    # b_attn / b_proj are zeros per the problem spec; not sent to the device.

    # LoRA folded: x@(W + s*B@A)^T  ==  x@W^T + s*(x@A^T)@B^T  exactly.
    W_attn_eff = W_attn + LORA_SCALE * (B_attn @ A_attn)
    W_proj_eff = W_proj + LORA_SCALE * (B_proj @ A_proj)

    kk = np.arange(KT)[:, None]
    qq = np.arange(TC)[None, :]
    masks = np.stack(
        [(qq >= kk + KT * j).astype(np.float32) for j in range(4)]
    )

    # one-hot head->dim broadcast matrix, live rows at partitions 32h:
    # e4[32h, p*128 + 64*h01 + d] = (h == 2p+h01)
    e4 = np.zeros((128, 2 * KT), np.float32)
    for h in range(HL):
        p, h01 = divmod(h, 2)
        e4[32 * h, p * 128 + 64 * h01: p * 128 + 64 * h01 + 64] = 1.0

    in_maps = []
    for core in range(8):
        b, m = divmod(core, TP)
        rs = slice(OQ * m, OQ * (m + 1))
        w_shard = np.concatenate(
            [W_attn_eff[rs], W_attn_eff[C:][rs], W_attn_eff[2 * C:][rs]], axis=0
        )
        cs = slice(CP * m, CP * (m + 1))
        in_maps.append({
            "xt": _bf(x[b].T),
            "wqkvt": _bf(w_shard.T),
            "wpt": _bf(W_proj_eff[:, cs].T),
            "masks": _bf(masks),
            "e4": _bf(e4),
        })
    return in_maps


def assemble(outs):
    y = np.zeros((B, T, C), np.float32)
    for g in range(B):
        yt = np.zeros((C, T), np.float32)
        for r in range(TP):
            yt += np.asarray(outs[TP * g + r], np.float32)
        y[g] = yt.T
    return y


_CACHE = {}


def run(inputs, trace=False):
    from concourse.bass_utils import run_bass_kernel_spmd

    if "nc" not in _CACHE:
        nc = build_nc()
        nc.compile()
        _CACHE["nc"] = nc
    res = run_bass_kernel_spmd(
        _CACHE["nc"], make_in_maps(inputs), core_ids=list(range(8)), trace=trace,
    )
    outs = [r["out"] for r in res.results]
    return assemble(outs), res


def kernel(**inputs):
    y, _ = run(inputs)
    return y


# revision 16
# speedup vs baseline: 1.0622x; 1.0622x over previous
"""Distributed Trainium2 kernel for causal multi-head attention with LoRA
(c_attn + c_proj both LoRA'd), B=2 T=2048 C=1024 H=16 hd=64 r=8.

Sharding: data-parallel over batch (2 groups of 4 cores) x tensor-parallel
over heads (4 heads / core).  Each core computes qkv for its heads, causal
attention, and a partial c_proj over its 256 input dims; the host sums the
4 partials per batch (collectives pay ~40us ncfw init here).

Host-side simplifications (all exact linear algebra, no approximation):
 - LoRA folds into the base weights: W_eff = W + LORA_SCALE * B @ A.
 - Everything is passed feature-major ("pre-transposed") so no on-device
   transposes are needed; the device output is y^T, transposed back on host.
 - b_attn / b_proj are zeros by the problem spec and are not applied.

Device compute is bf16 (fp32 PSUM accumulation; rel-err budget 2e-2).

Schedule (single merged emission stream, engines pipeline via Tile deps):
 - qkv production is t-chunk-major (A(ci)), and attention q-chunks are
   processed in causal order (256,512,1024,1536,0) so attention's exp
   (ScalarE-bound) overlaps qkv GEMMs (PE-bound) instead of serializing.
 - A-quanta (one qk j-tile / v t-tile GEMM chain) and c_proj m-tiles of the
   previous chunk are woven between attention windows from a filler queue,
   keeping the PE dense so the HAM clock gate stays at 8/8.
 - S^T tiles ([k,q] layout): the two heads of a pair are emitted
   interleaved so their K=64 matmuls row-pack (tile_position rows 0/64)
   and run concurrently on the PE.  Fully-masked columns (q < 128j on
   diagonal tiles) are skipped in both the S matmuls and the exp.
 - exp on ScalarE (scale=1/8 folded in; no max-subtraction: |logits|<~4,
   fp32 exp overflows at 88) -> causal 0/1 mask multiply on the 128-col
   diagonal block only -> PV matmul with V augmented by a ones column so
   softmax denominators fall out of the same matmul (psum row 64).
 - O is copied out unnormalized (frees PSUM); denominators are
   reciprocal'd on a packed [4, qw] tile and broadcast to the 64 dims of
   each head with a tiny K=4 one-hot matmul (E4).
"""

import numpy as np
import ml_dtypes

import concourse.bass as bass
import concourse.mybir as mybir
import concourse.tile as tile
from concourse import bacc

BF16 = mybir.dt.bfloat16
F32 = mybir.dt.float32
NPBF = ml_dtypes.bfloat16

B, T, C = 2, 2048, 1024
H, HD, R = 16, 64, 8
LORA_SCALE = 2.0

TP = 4                 # tensor-parallel ranks per batch group
HL = H // TP           # heads per core = 4
OQ = HL * HD           # local q rows = 256
OL = 3 * OQ            # local qkv rows = 768
CP = C // TP           # local c_proj contraction dims = 256
TC = 512               # t-chunk (matmul free dim)
NTC = T // TC          # 4 chunks
KT = 128               # k tile (partition dim of S^T)
NCT = C // 128         # 8 contraction tiles for c_attn

# attention q-chunks in causal-availability order; (q0, qw, ci_needed)
CHUNKS = [(256, 256, 0), (512, 512, 1), (1024, 512, 2), (1536, 512, 3),
          (0, 256, 0)]
N_WARM = 28            # PE warmup matmuls (cover the input-DMA window)


def build_nc():
    nc = bacc.Bacc(None, target_bir_lowering=False)

    xt_d = nc.declare_dram_parameter("xt", [C, T], BF16, isOutput=False)
    wqkvt_d = nc.declare_dram_parameter("wqkvt", [C, OL], BF16, isOutput=False)
    wpt_d = nc.declare_dram_parameter("wpt", [CP, C], BF16, isOutput=False)
    masks_d = nc.declare_dram_parameter("masks", [4, KT, TC], BF16, isOutput=False)
    e4_d = nc.declare_dram_parameter("e4", [128, 2 * KT], BF16, isOutput=False)
    out_d = nc.declare_dram_parameter("out", [C, T], BF16, isOutput=True)

    with tile.TileContext(nc) as tc:
        with (
            tc.tile_pool(name="const", bufs=1) as const,
            tc.tile_pool(name="work", bufs=3) as work,
            tc.tile_pool(name="ps_lin", bufs=2, space="PSUM") as ps_lin,
            tc.tile_pool(name="ps_s", bufs=2, space="PSUM") as ps_s,
            tc.tile_pool(name="ps_o", bufs=1, space="PSUM") as ps_o,
        ):
            # ---------------- persistent SBUF tensors ----------------
            warm_s = const.tile([128, TC], BF16, tag="warm")
            nc.gpsimd.memset(warm_s, 0.0)
            warm_ps = ps_o.tile([128, TC], F32, tag="o0", name="warm_ps")
            for _ in range(N_WARM):
                nc.tensor.matmul(
                    warm_ps, lhsT=warm_s[:, :128], rhs=warm_s,
                    start=True, stop=True,
                )

            wq_s = const.tile([128, NCT, OL], BF16, tag="wq")
            wq_r = wqkvt_d.rearrange("(n p) o -> p n o", p=128)
            for n in range(NCT):
                nc.sync.dma_start(out=wq_s[:, n, :], in_=wq_r[:, n, :])

            xt_s = const.tile([128, NCT, T], BF16, tag="xt")
            xt_r = xt_d.rearrange("(n p) t -> p n t", p=128)
            for ci in range(NTC):
                for n in range(NCT):
                    nc.sync.dma_start(
                        out=xt_s[:, n, bass.ts(ci, TC)],
                        in_=xt_r[:, n, bass.ts(ci, TC)],
                    )
                if ci == 0:
                    mask_s = const.tile([128, 4, TC], BF16, tag="mask")
                    nc.sync.dma_start(
                        out=mask_s, in_=masks_d.rearrange("j p q -> p j q")
                    )
                    e4_s = const.tile([128, 2 * KT], BF16, tag="e4")
                    nc.sync.dma_start(out=e4_s, in_=e4_d[:, :])

            wpt_s = const.tile([128, CP // 128, C], BF16, tag="wpt")
            nc.sync.dma_start(out=wpt_s, in_=wpt_d.rearrange("(n p) o -> p n o", p=128))

            # q,k feature-major: tiles 0,1 = q (256 rows), 2,3 = k
            qkvt_s = const.tile([128, 4, T], BF16, tag="qkvt")
            # v token-major, augmented: per t-tile, 4 heads x (64 dims + ones)
            v_s = const.tile([128, T // 128, HL * (HD + 1)], BF16, tag="v")
            nc.vector.memset(v_s, 1.0)  # ones columns survive the V copies
            ot_s = const.tile([128, CP // 128, T], BF16, tag="ot")

            # ---------------- filler quanta (PE work to weave in) -----------
            def qk_quantum(ci, j):
                def emit():
                    osl = bass.ts(j, 128)
                    tsl = bass.ts(ci, TC)
                    qk_ps = ps_lin.tile([128, TC], F32, tag="lin", name="qk_ps")
                    for n in range(NCT):
                        nc.tensor.matmul(
                            qk_ps, lhsT=wq_s[:, n, osl], rhs=xt_s[:, n, tsl],
                            start=(n == 0), stop=(n == NCT - 1),
                        )
                    nc.vector.tensor_copy(qkvt_s[:, j, tsl], qk_ps)
                return emit

            def v_quantum(tt):
                def emit():
                    v_ps = ps_lin.tile([128, TC], F32, tag="lin", name="v_ps")
                    ttsl = bass.ts(tt, 128)
                    for n in range(NCT):
                        nc.tensor.matmul(
                            v_ps[:, :OQ], lhsT=xt_s[:, n, ttsl],
                            rhs=wq_s[:, n, 2 * OQ:OL],
                            start=(n == 0), stop=(n == NCT - 1),
                        )
                    dst = v_s[:, tt, :].rearrange(
                        "p (h e) -> p h e", e=HD + 1)[:, :, 0:HD]
                    nc.vector.tensor_copy(
                        dst, v_ps[:, :OQ].rearrange("p (h e) -> p h e", e=HD)
                    )
                return emit

            # filler queue: (ci_tag, emit_fn); ci_tag = qkv chunk this quantum
            # belongs to (for barrier drains), or None for np work.
            fillers = []
            for ci in range(NTC):
                for j in range(4):
                    fillers.append((ci, qk_quantum(ci, j)))
                for tt in range(4 * ci, 4 * ci + 4):
                    fillers.append((ci, v_quantum(tt)))

            def pop_filler(k):
                for _ in range(k):
                    if fillers:
                        fillers.pop(0)[1]()

            def drain_through(ci):
                # emit every queued quantum up to and including the last one
                # tagged <= ci (np fillers in between run too; they're ready)
                last = -1
                for i, (tag, _) in enumerate(fillers):
                    if tag is not None and tag <= ci:
                        last = i
                if last >= 0:
                    for _, emit in fillers[:last + 1]:
                        emit()
                    del fillers[:last + 1]

            # ---------------- attention ----------------
            def attn_chunk(q0, qw):
                tsl = slice(q0, q0 + qw)
                kt0 = q0 // 128
                nkt = kt0 + qw // 128   # causal k-tiles for this chunk
                sums4 = work.tile([128, TC], F32, tag="sums", name="sums",
                                  bufs=2)
                # junk rows must stay finite: Ln runs on all 128 partitions
                nc.vector.memset(sums4[:, :qw], 1.0)
                for p in range(2):          # head pairs (2p, 2p+1)
                    o_ps = [
                        ps_o.tile([128, TC], F32, tag=f"o{h01}", name=f"o{h01}")
                        for h01 in range(2)
                    ]
                    for w in range(nkt // 2):   # windows of 2 k-tiles
                        # S matmuls, h0/h1 interleaved so the two K=64
                        # matmuls row-pack (rows 0-63 / 64-127) on the PE
                        s_ps = [
                            ps_s.tile([128, 2 * TC], F32, tag=f"s{h01}",
                                      name=f"s{h01}", bufs=1)
                            for h01 in range(2)
                        ]
                        qls = []
                        for kt01 in range(2):
                            kt = 2 * w + kt01
                            ql = max(0, 128 * (kt - kt0))
                            qls.append(ql)
                            for h01 in range(2):
                                dsl = slice(64 * h01, 64 * h01 + 64)
                                nc.tensor.matmul(
                                    s_ps[h01][:, kt01 * qw + ql:(kt01 + 1) * qw],
                                    lhsT=qkvt_s[dsl, 2 + p, bass.ts(kt, KT)],
                                    rhs=qkvt_s[dsl, p, q0 + ql:q0 + qw],
                                    start=True, stop=True,
                                )
                        ql0, ql1 = qls
                        for h01 in range(2):
                            h = 2 * p + h01
                            pt = work.tile(
                                [128, 2 * TC], BF16, tag=f"pt{h01}",
                                name=f"pt{h01}", bufs=6,
                            )
                            if ql1 >= 352:
                                # two exp calls beat one (352-cycle ACT
                                # instruction overhead crossover)
                                for kt01, ql in enumerate(qls):
                                    nc.scalar.activation(
                                        pt[:, kt01 * qw + ql:(kt01 + 1) * qw],
                                        s_ps[h01][:, kt01 * qw + ql:(kt01 + 1) * qw],
                                        mybir.ActivationFunctionType.Exp,
                                        scale=0.125,
                                    )
                            else:
                                # single call; the dead gap [qw, qw+ql1) is
                                # exp'd on stale psum and never read
                                nc.scalar.activation(
                                    pt[:, ql0:2 * qw],
                                    s_ps[h01][:, ql0:2 * qw],
                                    mybir.ActivationFunctionType.Exp,
                                    scale=0.125,
                                )
                            # causal masking: only the 128-col diagonal block
                            # of diagonal tiles is partially masked
                            for kt01 in range(2):
                                j = 2 * w + kt01 - kt0
                                if j >= 0:
                                    c0 = 128 * j
                                    c1 = min(c0 + 128, qw)
                                    nc.vector.tensor_mul(
                                        pt[:, kt01 * qw + c0:kt01 * qw + c1],
                                        pt[:, kt01 * qw + c0:kt01 * qw + c1],
                                        mask_s[:, j, c0:c1],
                                    )
                            for kt01 in range(2):
                                kt = 2 * w + kt01
                                ql = qls[kt01]
                                nc.tensor.matmul(
                                    o_ps[h01][: HD + 1, ql:qw],
                                    lhsT=v_s[:, kt, h * (HD + 1):(h + 1) * (HD + 1)],
                                    rhs=pt[:, kt01 * qw + ql:(kt01 + 1) * qw],
                                    start=(kt == 0),
                                    stop=(kt == nkt - 1),
                                )
                        pop_filler(2)
                    # copy O out unnormalized (frees psum); gather denominators
                    for h01 in range(2):
                        h = 2 * p + h01
                        nc.vector.tensor_copy(
                            ot_s[64 * h01:64 * h01 + 64, p, tsl],
                            o_ps[h01][0:HD, :qw],
                        )
                        nc.vector.tensor_copy(
                            sums4[32 * h:32 * h + 1, :qw],
                            o_ps[h01][HD:HD + 1, :qw],
                        )
                return sums4

            # ---------------- normalize + c_proj (as filler quanta) ---------
            def np_recip(qw, sums4):
                # 1/d = exp(-ln d) on ScalarE: the DVE reciprocal instruction
                # is free-dim-serial (~5.25 ns/col); ACT does this in 2 passes
                # at 1 elem/cycle/partition.  Emitted inline (ACT-only work).
                lnd = work.tile([128, TC], F32, tag="lnd", name="lnd", bufs=2)
                recip4 = work.tile([128, TC], BF16, tag="recip", name="recip",
                                   bufs=2)
                nc.scalar.activation(
                    lnd[:, :qw], sums4[:, :qw], mybir.ActivationFunctionType.Ln
                )
                nc.scalar.activation(
                    recip4[:, :qw], lnd[:, :qw],
                    mybir.ActivationFunctionType.Exp, scale=-1.0,
                )
                return recip4

            def np_bcast(q0, qw, recip4):
                def emit():
                    tsl = slice(q0, q0 + qw)
                    for p in range(2):
                        rb_ps = ps_lin.tile([128, TC], F32, tag="lin",
                                            name="rb_ps")
                        nc.tensor.matmul(
                            rb_ps[:, :qw], lhsT=e4_s[:, bass.ts(p, 128)],
                            rhs=recip4[:, :qw], start=True, stop=True,
                        )
                        dst = ot_s[:, p, tsl]
                        nc.vector.tensor_mul(dst, dst, rb_ps[:, :qw])
                return emit

            def np_mtile(q0, qw, m, yt_sb, tail):
                def emit():
                    tsl = slice(q0, q0 + qw)
                    msl = bass.ts(m, 128)
                    y_ps = ps_lin.tile([128, TC], F32, tag="lin", name="y_ps")
                    for n in range(CP // 128):
                        nc.tensor.matmul(
                            y_ps[:, :qw], lhsT=wpt_s[:, n, msl],
                            rhs=ot_s[:, n, tsl],
                            start=(n == 0), stop=(n == CP // 128 - 1),
                        )
                    if tail:
                        nc.scalar.copy(yt_sb[:, m, :qw], y_ps[:, :qw])
                    else:
                        nc.vector.tensor_copy(yt_sb[:, m, :qw], y_ps[:, :qw])
                    if m in (3, 7):
                        out_r = out_d.rearrange("(m p) t -> p m t", p=128)
                        nc.sync.dma_start(
                            out=out_r[:, m - 3:m + 1, tsl],
                            in_=yt_sb[:, m - 3:m + 1, :qw],
                        )
                return emit

            def queue_np(q0, qw, sums4, tail=False):
                recip4 = np_recip(qw, sums4)
                yt_sb = work.tile([128, C // 128, TC], BF16, tag="yt", bufs=2,
                                  name="yt")
                fillers.append((None, np_bcast(q0, qw, recip4)))
                for m in range(C // 128):
                    fillers.append((None, np_mtile(q0, qw, m, yt_sb, tail)))

            # ---------------- main schedule ----------------
            for idx, (q0, qw, ci_need) in enumerate(CHUNKS):
                drain_through(ci_need)
                sums4 = attn_chunk(q0, qw)
                queue_np(q0, qw, sums4, tail=(idx >= 3))
            for _, emit in fillers:
                emit()
            fillers.clear()

    return nc


# ---------------- host side ----------------

def _bf(a):
    return np.ascontiguousarray(np.asarray(a, dtype=np.float32).astype(NPBF))


def make_in_maps(inputs):
    x = np.asarray(inputs["x"], np.float32)
    W_attn = np.asarray(inputs["W_attn"], np.float32)
    A_attn = np.asarray(inputs["A_attn"], np.float32)
    B_attn = np.asarray(inputs["B_attn"], np.float32)
    W_proj = np.asarray(inputs["W_proj"], np.float32)
    A_proj = np.asarray(inputs["A_proj"], np.float32)
    B_proj = np.asarray(inputs["B_proj"], np.float32)
    # b_attn / b_proj are zeros per the problem spec; not sent to the device.

    # LoRA folded: x@(W + s*B@A)^T  ==  x@W^T + s*(x@A^T)@B^T  exactly.
    W_attn_eff = W_attn + LORA_SCALE * (B_attn @ A_attn)
    W_proj_eff = W_proj + LORA_SCALE * (B_proj @ A_proj)

    kk = np.arange(KT)[:, None]
    qq = np.arange(TC)[None, :]
    masks = np.stack(
        [(qq >= kk + KT * j).astype(np.float32) for j in range(4)]
    )

    # one-hot head->dim broadcast matrix, live rows at partitions 32h:
    # e4[32h, p*128 + 64*h01 + d] = (h == 2p+h01)
    e4 = np.zeros((128, 2 * KT), np.float32)
    for h in range(HL):
        p, h01 = divmod(h, 2)
        e4[32 * h, p * 128 + 64 * h01: p * 128 + 64 * h01 + 64] = 1.0

    in_maps = []
    for core in range(8):
        b, m = divmod(core, TP)
        rs = slice(OQ * m, OQ * (m + 1))
        w_shard = np.concatenate(
            [W_attn_eff[rs], W_attn_eff[C:][rs], W_attn_eff[2 * C:][rs]], axis=0
        )
        cs = slice(CP * m, CP * (m + 1))
        in_maps.append({
            "xt": _bf(x[b].T),
            "wqkvt": _bf(w_shard.T),
            "wpt": _bf(W_proj_eff[:, cs].T),
            "masks": _bf(masks),
            "e4": _bf(e4),
        })
    return in_maps


def assemble(outs):
    y = np.zeros((B, T, C), np.float32)
    for g in range(B):
        yt = np.zeros((C, T), np.float32)
        for r in range(TP):
            yt += np.asarray(outs[TP * g + r], np.float32)
        y[g] = yt.T
    return y


_CACHE = {}


def run(inputs, trace=False):
    from concourse.bass_utils import run_bass_kernel_spmd

    if "nc" not in _CACHE:
        nc = build_nc()
        nc.compile()
        _CACHE["nc"] = nc
    res = run_bass_kernel_spmd(
        _CACHE["nc"], make_in_maps(inputs), core_ids=list(range(8)), trace=trace,
    )
    outs = [r["out"] for r in res.results]
    return assemble(outs), res


def kernel(**inputs):
    y, _ = run(inputs)
    return y
